# revision 4
# baseline (speedup 1.0000x reference)
"""Trainium2 Bass kernel for nn_ComplexPointNetwork (gnn_message_passing).

Key insight: the KNN gather / neighbor-max path in the reference is dead code
(`xcat[:, :H]` slices back exactly `x`), so `knn_idx`/`coord`/`offset` never
affect the output.  The real computation is a 5-layer MLP with train-mode
BatchNorm (statistics over the full N=120000 points) and one residual add:

    x1 = relu(bn1(feat @ w1.T))          # [N, 128]
    x2 = relu(bn2(x1 @ w2.T))            # [N, 128]   (identity)
    x3 = relu(bn3(x2 @ w3.T))            # [N, 256]
    x4 = bn4(x3 @ w4.T)                  # [N, 128]
    x5 = relu(x4 + x2)
    out = x5 @ w_out.T + b_out           # [N, 8]

Distribution: data-parallel over points (15000/core on 8 cores), with tiny
per-layer AllGathers of per-channel (sum, sumsq) partial statistics.

Device-side structure per BN layer (two-pass recompute):
  pass 1: matmul -> PSUM, DVE bn_stats per tile (stats only, y discarded)
  AllGather 1-2KB partial stats -> combine -> per-channel scale/bias vectors
  pass 2: matmul again -> PSUM, fused ScalarE `relu(scale*y + bias)` PSUM->SBUF

Activations live channel-major [C, points] in SBUF so layer weights are the
stationary matmul operand.  L1 stats are computed analytically from the 6x6
second-moment matrix of [feat | 1] (PE Gram accumulation), making L1
single-pass.  The L4 residual is folded into the pass-2 matmul accumulation
via a diag(1/scale4) matmul on x2, so the final fused ScalarE op computes
relu(scale4*(y4 + x2/scale4) + bias4) = relu(bn4(y4) + x2) exactly.
"""

import sys

if "/opt/trn_rl_repo" not in sys.path:
    sys.path.insert(0, "/opt/trn_rl_repo")

import numpy as np
import ml_dtypes

N = 120000
NCORES = 8
NS = N // NCORES            # 15000 real points per core
TILE_F = 512
NT = 30                     # free-dim tiles per core (padded)
NSP = NT * TILE_F           # 15360 padded points per core
NPT = NSP // 128            # 120 partition-tiles for the Gram phase
LAST_REAL = NS - (NT - 1) * TILE_F   # 152 real points in the last tile
C_IN = 5
H = 128
H2 = 256
C_OUT = 8
EPS = 1e-5

_CACHE = {}


def _build_program():
    import concourse.bass as bass
    import concourse.bacc as bacc
    import concourse.tile as tile
    from concourse import mybir
    from concourse.masks import make_identity

    f32 = mybir.dt.float32
    bf16 = mybir.dt.bfloat16
    AF = mybir.ActivationFunctionType
    OP = mybir.AluOpType

    nc = bacc.Bacc(
        "TRN2",
        target_bir_lowering=False,
        debug=False,
        enable_asserts=False,
        num_devices=NCORES,
    )

    featT_d = nc.dram_tensor("featT", [C_IN, NSP], f32, kind="ExternalInput")
    faug_d = nc.dram_tensor("faug", [NSP, 6], f32, kind="ExternalInput")
    w1T_d = nc.dram_tensor("w1T", [C_IN, H], f32, kind="ExternalInput")
    w2T_d = nc.dram_tensor("w2T", [H, H], bf16, kind="ExternalInput")
    w3T_d = nc.dram_tensor("w3T", [H, H2], f32, kind="ExternalInput")
    w4Ta_d = nc.dram_tensor("w4Ta", [H, H], bf16, kind="ExternalInput")
    w4Tb_d = nc.dram_tensor("w4Tb", [H, H], bf16, kind="ExternalInput")
    woutT_d = nc.dram_tensor("woutT", [H, C_OUT], bf16, kind="ExternalInput")
    gb_d = nc.dram_tensor("gb", [H, 10], f32, kind="ExternalInput")
    bout_d = nc.dram_tensor("bout", [C_OUT, 1], f32, kind="ExternalInput")
    outT_d = nc.dram_tensor("outT", [C_OUT, NSP], f32, kind="ExternalOutput")

    rg = [list(range(NCORES))]

    with tile.TileContext(nc) as tc:
        with (
            tc.tile_pool(name="acts16", bufs=90) as acts16,
            tc.tile_pool(name="acts32", bufs=30) as acts32,
            tc.tile_pool(name="x5p", bufs=4) as x5p,
            tc.tile_pool(name="outp", bufs=3) as outp,
            tc.tile_pool(name="wts", bufs=1) as wts,
            tc.tile_pool(name="featp", bufs=4) as featp,
            tc.tile_pool(name="stat", bufs=1) as stat,
            tc.tile_pool(name="psum_y", bufs=4, space="PSUM") as psum_y,
            tc.tile_pool(name="psum_g", bufs=1, space="PSUM") as psum_g,
            tc.tile_pool(name="psum_s", bufs=2, space="PSUM") as psum_s,
            tc.tile_pool(name="psum_o", bufs=1, space="PSUM") as psum_o,
            tc.tile_pool(name="dram", bufs=1, space="DRAM") as dram,
        ):
            # ---------------- load weights / constants ----------------
            w1T = wts.tile([C_IN, H], f32, tag="w1T")
            nc.sync.dma_start(out=w1T[:], in_=w1T_d.ap())
            w2T = wts.tile([H, H], bf16, tag="w2T")
            nc.sync.dma_start(out=w2T[:], in_=w2T_d.ap())
            w3T = wts.tile([H, H2], f32, tag="w3T")
            nc.sync.dma_start(out=w3T[:], in_=w3T_d.ap())
            w4Ta = wts.tile([H, H], bf16, tag="w4Ta")
            nc.sync.dma_start(out=w4Ta[:], in_=w4Ta_d.ap())
            w4Tb = wts.tile([H, H], bf16, tag="w4Tb")
            nc.sync.dma_start(out=w4Tb[:], in_=w4Tb_d.ap())
            woutT = wts.tile([H, C_OUT], bf16, tag="woutT")
            nc.sync.dma_start(out=woutT[:], in_=woutT_d.ap())
            gb = wts.tile([H, 10], f32, tag="gb")
            nc.sync.dma_start(out=gb[:], in_=gb_d.ap())
            bout = wts.tile([C_OUT, 1], f32, tag="bout")
            nc.sync.dma_start(out=bout[:], in_=bout_d.ap())
            i128 = wts.tile([H, H], f32, tag="i128")
            make_identity(nc, i128[:])

            def sb(shape, tag, dt=f32):
                return stat.tile(shape, dt, tag=tag, name=tag)

            eps_t = sb([H, 1], "eps_t")
            nc.vector.memset(eps_t[:], EPS)

            # helper: from global (sum, sqsum) [C,1] fp32 in SBUF produce
            # scale = g/sqrt(var+eps), bias = beta - mean*scale   (C<=128)
            def scale_bias(sum_sb, sq_sb, g_ap, b_ap, tag, cnt=float(N)):
                c = sum_sb.shape[0]
                negmean = sb([c, 1], f"negmean{tag}")
                nc.vector.tensor_scalar_mul(out=negmean[:], in0=sum_sb, scalar1=-1.0 / cnt)
                ey2 = sb([c, 1], f"ey2{tag}")
                nc.vector.tensor_scalar_mul(out=ey2[:], in0=sq_sb, scalar1=1.0 / cnt)
                m2 = sb([c, 1], f"m2{tag}")
                nc.vector.tensor_mul(out=m2[:], in0=negmean[:], in1=negmean[:])
                var = sb([c, 1], f"var{tag}")
                nc.vector.tensor_sub(out=var[:], in0=ey2[:], in1=m2[:])
                sd = sb([c, 1], f"sd{tag}")
                nc.scalar.activation(
                    out=sd[:], in_=var[:], func=AF.Sqrt, bias=eps_t[0:c, :]
                )
                rstd = sb([c, 1], f"rstd{tag}")
                nc.vector.reciprocal(out=rstd[:], in_=sd[:])
                scale = sb([c, 1], f"scale{tag}")
                nc.vector.tensor_mul(out=scale[:], in0=g_ap, in1=rstd[:])
                tmp = sb([c, 1], f"tmp{tag}")
                nc.vector.tensor_mul(out=tmp[:], in0=negmean[:], in1=scale[:])
                bias = sb([c, 1], f"bias{tag}")
                nc.vector.tensor_add(out=bias[:], in0=b_ap, in1=tmp[:])
                return scale, bias

            # ---------------- phase 0: Gram of [feat | 1] ----------------
            gram_ps = psum_g.tile([6, 6], f32)
            for i in range(NPT):
                fa = featp.tile([128, 6], f32, tag="fa")
                nc.sync.dma_start(out=fa[:], in_=faug_d.ap()[i * 128:(i + 1) * 128, :])
                nc.tensor.matmul(
                    out=gram_ps[:], lhsT=fa[:], rhs=fa[:],
                    start=(i == 0), stop=(i == NPT - 1),
                )
            gram_l = sb([6, 6], "gram_l")
            nc.vector.tensor_copy(out=gram_l[:], in_=gram_ps[:])

            # AllGather the local 6x6 gram, combine to global
            gin = dram.tile([6, 6], f32, tag="gin")
            gout = dram.tile([NCORES, 6, 6], f32, tag="gout")
            nc.sync.dma_start(out=gin[:], in_=gram_l[:])
            nc.gpsimd.collective_compute(
                "AllGather", OP.bypass, replica_groups=rg,
                ins=[gin.opt()], outs=[gout.opt()],
            )
            gall = sb([6, 6, NCORES], "gall")   # partition=row, free=(col, rank)
            nc.sync.dma_start(out=gall[:], in_=gout[:].rearrange("r c j -> c j r"))
            gram = sb([6, 6], "gram")
            nc.vector.tensor_reduce(
                out=gram[:], in_=gall[:], axis=mybir.AxisListType.X, op=OP.add,
            )

            # L1 stats from gram: sum_y1 = w1 @ sumf ; sq1_j = w1_j S w1_j^T
            sumf = gram[0:C_IN, 5:6]
            S = gram[0:C_IN, 0:C_IN]
            s1_ps = psum_s.tile([H, 1], f32, tag="ps_small")
            nc.tensor.matmul(out=s1_ps[:], lhsT=w1T[:], rhs=sumf)
            sum1 = sb([H, 1], "sum1")
            nc.vector.tensor_copy(out=sum1[:], in_=s1_ps[:])

            a_ps = psum_s.tile([C_IN, H], f32, tag="ps_small")
            nc.tensor.matmul(out=a_ps[:], lhsT=S, rhs=w1T[:])
            bmat = sb([C_IN, H], "bmat")
            nc.vector.tensor_mul(out=bmat[:], in0=w1T[:], in1=a_ps[:])
            ones5 = sb([C_IN, 1], "ones5")
            nc.vector.memset(ones5[:], 1.0)
            sqrow_ps = psum_s.tile([1, H], f32, tag="ps_small")
            nc.tensor.matmul(out=sqrow_ps[:], lhsT=ones5[:], rhs=bmat[:])
            sqrow = sb([1, H], "sqrow")
            nc.vector.tensor_copy(out=sqrow[:], in_=sqrow_ps[:])
            ones1 = sb([1, 1], "ones1")
            nc.vector.memset(ones1[:], 1.0)
            sq1_ps = psum_s.tile([H, 1], f32, tag="ps_small")
            nc.tensor.matmul(out=sq1_ps[:], lhsT=sqrow[:], rhs=ones1[:])
            sq1 = sb([H, 1], "sq1")
            nc.vector.tensor_copy(out=sq1[:], in_=sq1_ps[:])

            scale1, bias1 = scale_bias(sum1[:], sq1[:], gb[:, 0:1], gb[:, 1:2], "1")

            # ---------------- L1 single pass ----------------
            x1 = []
            for t in range(NT):
                ft = featp.tile([C_IN, TILE_F], f32, tag="ft")
                nc.sync.dma_start(
                    out=ft[:], in_=featT_d.ap()[:, t * TILE_F:(t + 1) * TILE_F]
                )
                yp = psum_y.tile([H, TILE_F], f32, tag="yp")
                nc.tensor.matmul(out=yp[:], lhsT=w1T[:], rhs=ft[:])
                xt = acts16.tile([H, TILE_F], bf16, tag="a16")
                nc.scalar.activation(
                    out=xt[:], in_=yp[:], func=AF.Relu, bias=bias1[:], scale=scale1[:]
                )
                x1.append(xt)

            # generic BN layer pass-1 stats + exchange (C<=128 per group)
            def bn_exchange(packs, tag):
                """packs: list of (mv_tile,) per group; AllGather (sum,sq) pairs.
                Returns list of (sum_sb, sq_sb) per group."""
                ng = len(packs)
                pk = sb([H, 2 * ng], f"pack{tag}")
                for gi, mv in enumerate(packs):
                    # sum = mean*NS ; sq = (var + mean^2)*NS   (real count NS)
                    nc.vector.tensor_scalar_mul(
                        out=pk[:, 2 * gi:2 * gi + 1], in0=mv[:, 0:1], scalar1=float(NS)
                    )
                    msq = sb([H, 1], f"msq{tag}{gi}")
                    nc.vector.tensor_mul(out=msq[:], in0=mv[:, 0:1], in1=mv[:, 0:1])
                    vps = sb([H, 1], f"vps{tag}{gi}")
                    nc.vector.tensor_add(out=vps[:], in0=mv[:, 1:2], in1=msq[:])
                    nc.vector.tensor_scalar_mul(
                        out=pk[:, 2 * gi + 1:2 * gi + 2], in0=vps[:], scalar1=float(NS)
                    )
                cin = dram.tile([H, 2 * ng], f32, tag=f"cin{tag}")
                cout = dram.tile([NCORES, H, 2 * ng], f32, tag=f"cout{tag}")
                nc.sync.dma_start(out=cin[:], in_=pk[:])
                nc.gpsimd.collective_compute(
                    "AllGather", OP.bypass, replica_groups=rg,
                    ins=[cin.opt()], outs=[cout.opt()],
                )
                allst = sb([H, 2 * ng, NCORES], f"allst{tag}")
                nc.sync.dma_start(
                    out=allst[:], in_=cout[:].rearrange("r c j -> c j r")
                )
                res = []
                for gi in range(ng):
                    gsum = sb([H, 1], f"gsum{tag}{gi}")
                    nc.vector.tensor_reduce(
                        out=gsum[:], in_=allst[:, 2 * gi, :],
                        axis=mybir.AxisListType.X, op=OP.add,
                    )
                    gsq = sb([H, 1], f"gsq{tag}{gi}")
                    nc.vector.tensor_reduce(
                        out=gsq[:], in_=allst[:, 2 * gi + 1, :],
                        axis=mybir.AxisListType.X, op=OP.add,
                    )
                    res.append((gsum, gsq))
                return res

            # ---------------- L2 ----------------
            stats2 = sb([H, NT, 6], "stats2")
            for t in range(NT):
                yp = psum_y.tile([H, TILE_F], f32, tag="yp")
                nc.tensor.matmul(out=yp[:], lhsT=w2T[:], rhs=x1[t][:])
                fsz = TILE_F if t < NT - 1 else LAST_REAL
                nc.vector.bn_stats(out=stats2[:, t, :], in_=yp[:, 0:fsz])
            mv2 = sb([H, 2], "mv2")
            nc.vector.bn_aggr(out=mv2[:], in_=stats2[:])
            (st2,) = bn_exchange([mv2], "2")
            scale2, bias2 = scale_bias(st2[0][:], st2[1][:], gb[:, 2:3], gb[:, 3:4], "2")

            x2 = []
            for t in range(NT):
                yp = psum_y.tile([H, TILE_F], f32, tag="yp")
                nc.tensor.matmul(out=yp[:], lhsT=w2T[:], rhs=x1[t][:])
                xt = acts32.tile([H, TILE_F], f32, tag="a32")
                nc.scalar.activation(
                    out=xt[:], in_=yp[:], func=AF.Relu, bias=bias2[:], scale=scale2[:]
                )
                x2.append(xt)

            # ---------------- L3 (256 channels = 2 groups) ----------------
            stats3a = sb([H, NT, 6], "stats3a")
            stats3b = sb([H, NT, 6], "stats3b")
            for t in range(NT):
                fsz = TILE_F if t < NT - 1 else LAST_REAL
                ypa = psum_y.tile([H, TILE_F], f32, tag="yp")
                nc.tensor.matmul(out=ypa[:], lhsT=w3T[:, 0:H], rhs=x2[t][:])
                nc.vector.bn_stats(out=stats3a[:, t, :], in_=ypa[:, 0:fsz])
                ypb = psum_y.tile([H, TILE_F], f32, tag="yp")
                nc.tensor.matmul(out=ypb[:], lhsT=w3T[:, H:H2], rhs=x2[t][:])
                nc.vector.bn_stats(out=stats3b[:, t, :], in_=ypb[:, 0:fsz])
            mv3a = sb([H, 2], "mv3a")
            nc.vector.bn_aggr(out=mv3a[:], in_=stats3a[:])
            mv3b = sb([H, 2], "mv3b")
            nc.vector.bn_aggr(out=mv3b[:], in_=stats3b[:])
            (st3a, st3b) = bn_exchange([mv3a, mv3b], "3")
            scale3a, bias3a = scale_bias(st3a[0][:], st3a[1][:], gb[:, 4:5], gb[:, 5:6], "3a")
            scale3b, bias3b = scale_bias(st3b[0][:], st3b[1][:], gb[:, 6:7], gb[:, 7:8], "3b")

            x3a, x3b = [], []
            for t in range(NT):
                ypa = psum_y.tile([H, TILE_F], f32, tag="yp")
                nc.tensor.matmul(out=ypa[:], lhsT=w3T[:, 0:H], rhs=x2[t][:])
                xa = acts16.tile([H, TILE_F], bf16, tag="a16")
                nc.scalar.activation(
                    out=xa[:], in_=ypa[:], func=AF.Relu, bias=bias3a[:], scale=scale3a[:]
                )
                x3a.append(xa)
                ypb = psum_y.tile([H, TILE_F], f32, tag="yp")
                nc.tensor.matmul(out=ypb[:], lhsT=w3T[:, H:H2], rhs=x2[t][:])
                xb = acts16.tile([H, TILE_F], bf16, tag="a16")
                nc.scalar.activation(
                    out=xb[:], in_=ypb[:], func=AF.Relu, bias=bias3b[:], scale=scale3b[:]
                )
                x3b.append(xb)

            # ---------------- L4 (K=256, residual folded) ----------------
            stats4 = sb([H, NT, 6], "stats4")
            for t in range(NT):
                fsz = TILE_F if t < NT - 1 else LAST_REAL
                yp = psum_y.tile([H, TILE_F], f32, tag="yp")
                nc.tensor.matmul(out=yp[:], lhsT=w4Ta[:], rhs=x3a[t][:], start=True, stop=False)
                nc.tensor.matmul(out=yp[:], lhsT=w4Tb[:], rhs=x3b[t][:], start=False, stop=True)
                nc.vector.bn_stats(out=stats4[:, t, :], in_=yp[:, 0:fsz])
            mv4 = sb([H, 2], "mv4")
            nc.vector.bn_aggr(out=mv4[:], in_=stats4[:])
            (st4,) = bn_exchange([mv4], "4")
            scale4, bias4 = scale_bias(st4[0][:], st4[1][:], gb[:, 8:9], gb[:, 9:10], "4")
            inv_s4 = sb([H, 1], "inv_s4")
            nc.vector.reciprocal(out=inv_s4[:], in_=scale4[:])
            diagm = sb([H, H], "diagm")
            nc.vector.tensor_scalar_mul(out=diagm[:], in0=i128[:], scalar1=inv_s4[:])

            for t in range(NT):
                yp = psum_y.tile([H, TILE_F], f32, tag="yp")
                nc.tensor.matmul(out=yp[:], lhsT=w4Ta[:], rhs=x3a[t][:], start=True, stop=False)
                nc.tensor.matmul(out=yp[:], lhsT=w4Tb[:], rhs=x3b[t][:], start=False, stop=False)
                nc.tensor.matmul(out=yp[:], lhsT=diagm[:], rhs=x2[t][:], start=False, stop=True)
                x5t = x5p.tile([H, TILE_F], bf16, tag="x5")
                nc.scalar.activation(
                    out=x5t[:], in_=yp[:], func=AF.Relu, bias=bias4[:], scale=scale4[:]
                )
                # ---- output layer, fused per tile ----
                op_ps = psum_o.tile([C_OUT, TILE_F], f32, tag="op")
                nc.tensor.matmul(out=op_ps[:], lhsT=woutT[:], rhs=x5t[:])
                ot = outp.tile([C_OUT, TILE_F], f32, tag="ot")
                nc.scalar.activation(
                    out=ot[:], in_=op_ps[:], func=AF.Identity, bias=bout[:], scale=1.0
                )
                nc.sync.dma_start(
                    out=outT_d.ap()[:, t * TILE_F:(t + 1) * TILE_F], in_=ot[:]
                )

    nc.compile()
    return nc


def _get_program():
    if "nc" not in _CACHE:
        _CACHE["nc"] = _build_program()
    return _CACHE["nc"]


def make_in_maps(feat, w1, g1, b1, w2, g2, b2, w3, g3, b3, w4, g4, b4, w_out, b_out):
    bf16 = ml_dtypes.bfloat16
    f32 = np.float32

    w1T = np.ascontiguousarray(np.asarray(w1, f32).T)              # [5,128]
    w2T = np.ascontiguousarray(np.asarray(w2, f32).T.astype(bf16))  # [128,128]
    w3T = np.ascontiguousarray(np.asarray(w3, f32).T)              # [128,256]
    w4T = np.asarray(w4, f32).T                                     # [256,128]
    w4Ta = np.ascontiguousarray(w4T[:H].astype(bf16))
    w4Tb = np.ascontiguousarray(w4T[H:].astype(bf16))
    woutT = np.ascontiguousarray(np.asarray(w_out, f32).T.astype(bf16))  # [128,8]
    gbm = np.zeros((H, 10), f32)
    for i, v in enumerate([g1, b1, g2, b2]):
        gbm[:, i] = np.asarray(v, f32)
    gbm[:, 4] = np.asarray(g3, f32)[:H]
    gbm[:, 5] = np.asarray(b3, f32)[:H]
    gbm[:, 6] = np.asarray(g3, f32)[H:]
    gbm[:, 7] = np.asarray(b3, f32)[H:]
    gbm[:, 8] = np.asarray(g4, f32)
    gbm[:, 9] = np.asarray(b4, f32)
    boutm = np.ascontiguousarray(np.asarray(b_out, f32).reshape(C_OUT, 1))

    feat = np.asarray(feat, f32)
    in_maps = []
    for c in range(NCORES):
        sl = feat[c * NS:(c + 1) * NS]                 # [15000, 5]
        featT = np.zeros((C_IN, NSP), f32)
        featT[:, :NS] = sl.T
        faug = np.zeros((NSP, 6), f32)
        faug[:NS, :C_IN] = sl
        faug[:, 5] = 1.0
        in_maps.append(dict(
            featT=featT, faug=faug, w1T=w1T, w2T=w2T, w3T=w3T,
            w4Ta=w4Ta, w4Tb=w4Tb, woutT=woutT, gb=gbm, bout=boutm,
        ))
    return in_maps


def assemble_output(results):
    return np.ascontiguousarray(
        np.concatenate([results[c]["outT"][:, :NS] for c in range(NCORES)], axis=1).T
    ).astype(np.float32)


def kernel(**inputs):
    from concourse import bass_utils

    nc = _get_program()
    in_maps = make_in_maps(
        inputs["feat"], inputs["w1"], inputs["g1"], inputs["b1"],
        inputs["w2"], inputs["g2"], inputs["b2"], inputs["w3"], inputs["g3"],
        inputs["b3"], inputs["w4"], inputs["g4"], inputs["b4"],
        inputs["w_out"], inputs["b_out"],
    )
    res = bass_utils.run_bass_kernel_spmd(nc, in_maps, core_ids=list(range(NCORES)))
    return assemble_output(res.results)
